# revision 1
# baseline (speedup 1.0000x reference)
# Performer (FAVOR+) attention, distributed over 8 Trainium2 NeuronCores.
#
# Strategy: data-parallel over the batch dimension (B=8 -> 1 batch element
# per core), weights replicated, per the sharding hint. FAVOR+ is computed
# exactly as the reference (per-row max for queries, per-head global max for
# keys, eps floor) in fp32. The whole per-batch pipeline is one fused XLA
# program compiled for the NeuronCore via the PJRT backend, executed SPMD
# across the 8 devices with jax.pmap.
import functools

import jax
import jax.numpy as jnp
import numpy as np

H = 12            # heads
NB_FEATURES = 256  # Performer random features
EPS = 1e-4

B, N, D = 8, 2048, 768
d = D // H


def _softmax_kernel(x, proj, is_query):
    # x: [h,n,d], proj: [m,d] -> phi(x): [h,n,m]
    dn = d ** -0.25
    ratio = proj.shape[0] ** -0.5
    xp = jnp.einsum('hnd,md->hnm', x * dn, proj)
    diag = jnp.sum(x * x, axis=-1, keepdims=True) * 0.5 * (dn ** 2)
    if is_query:
        m = jax.lax.stop_gradient(jnp.max(xp, axis=-1, keepdims=True))
    else:
        m = jax.lax.stop_gradient(jnp.max(xp, axis=(-1, -2), keepdims=True))
    return ratio * (jnp.exp(xp - diag - m) + EPS)


def _one_batch(x, Wq, bq, Wk, bk, Wv, bv, Wo, bo, proj):
    # x: [N, D] (one batch element, on one core)
    n, dim = x.shape
    q = (x @ Wq.T + bq).reshape(n, H, d).transpose(1, 0, 2)
    k = (x @ Wk.T + bk).reshape(n, H, d).transpose(1, 0, 2)
    v = (x @ Wv.T + bv).reshape(n, H, d).transpose(1, 0, 2)
    qp = _softmax_kernel(q, proj, True)
    kp = _softmax_kernel(k, proj, False)
    k_sum = kp.sum(axis=1)                                # [h,m]
    D_inv = 1.0 / jnp.einsum('hnm,hm->hn', qp, k_sum)     # [h,n]
    ctx = jnp.einsum('hnm,hnd->hmd', kp, v)               # [h,m,d]
    out = jnp.einsum('hmd,hnm,hn->hnd', ctx, qp, D_inv)
    out = out.transpose(1, 0, 2).reshape(n, dim)
    return out @ Wo.T + bo


_PMAPPED = None


def _get_pmapped():
    global _PMAPPED
    if _PMAPPED is None:
        _PMAPPED = jax.pmap(
            _one_batch,
            in_axes=(0, None, None, None, None, None, None, None, None, None),
        )
    return _PMAPPED


def kernel(x, Wq, bq, Wk, bk, Wv, bv, Wo, bo, proj):
    x = np.asarray(x, dtype=np.float32)
    args = [np.asarray(a, dtype=np.float32)
            for a in (Wq, bq, Wk, bk, Wv, bv, Wo, bo, proj)]
    f = _get_pmapped()
    out = f(x, *args)                    # [B, N, D] sharded over 8 cores
    return np.asarray(out).astype(np.float32)
